# revision 11
# baseline (speedup 1.0000x reference)
"""GCN (2x GCNConv + global_add_pool + Linear) on 8 Trainium2 NeuronCores.

Strategy (edge-cut data parallel, hardcoded for N=100000, E=1600000, F=128,
OUT=64, G=512, 8 cores):

  * Symmetric normalization factorizes: norm = dinv[src]*dinv[dst], so we
    pre-scale the gather table by dinv and post-scale the aggregate by dinv.
  * The linear transform commutes with aggregation:
    segsum(x[src]) @ W == segsum((x@W)[src]), so each layer is
    SpMM(gather+segsum) -> small local matmul -> bias -> relu.
  * Nodes are partitioned contiguously across the 8 cores (12500 each);
    every message (edge or self-loop) is processed by the core owning its
    dst. Messages are gathered from a replicated fp16 node table in DRAM
    with dma_gather (int16 indices => 4 stride-4 "classes" of table rows).
  * Aggregation happens on-chip: messages land in SBUF tiles of 128, a
    one-hot [msg x dst-slot] fp16 matrix is built on VectorE (is_equal vs
    an iota), and TensorE accumulates agg^T[f, dst] in PSUM per 128-node
    window. Cells (window x class) have a static capacity of 640 slots
    (5 tiles); pads gather row 0 and carry dstloc=-1 (no one-hot match).
  * Layer boundary: each core computes its 12500-row slice of the next
    pre-scaled table; AllGather replicates it for the next layer's gather.
  * Pooling: per window, one-hot [node x graph] matmul accumulates
    pooled^T[f, g] in PSUM; AllReduce over cores; final Linear is computed
    redundantly on every core.
"""

import numpy as np

N = 100000
E = 1600000
F = 128
OUT = 64
G = 512
P = 8
C = N // P            # 12500 nodes per core
CLS = 4               # table row classes (stride trick for int16 gather idx)
NPC = N // CLS        # rows per class view
WPC = 98              # windows per core (ceil(12500/128))
CPAD = WPC * 128      # 12544 padded nodes per core
LASTW = C - (WPC - 1) * 128  # 84 real nodes in the last window
CAP = 640             # slots per (window, class) cell == 5 tiles
TPC = CAP // 128      # 5 tiles per cell
TPW = CLS * TPC       # 20 tiles per window
SEGW = 7              # windows per gather segment
NSEG = WPC // SEGW    # 14 segments
SEGIDX = SEGW * CAP   # 4480 gather idxs per (class, segment)
SEGT = SEGIDX // 128  # 35 tiles per (class, segment)
SEGC = SEGIDX // 16   # 280 idx columns per (class, segment)
CLSIDX = WPC * CAP    # 62720 idxs per class stream
CLSC = CLSIDX // 16   # 3920 idx columns per class stream
NTGT = WPC * TPW      # 1960 one-hot target columns per layer

_cache = {}


def _wrap_idx(idx):
    """[n] -> [128, n//16] int16, wrapped in 16 partitions, replicated x8."""
    n = idx.shape[0]
    w = idx.reshape(n // 16, 16).T.astype(np.int16)
    return np.tile(w, (P, 1))


def _assign(src, dst):
    """Choose node relabeling r (new id per node) s.t. every
    (core-window, class) cell count <= CAP. Identity normally works for
    this problem's statistics; a class-preserving swap repair handles the
    tail if needed."""
    rng = np.random.default_rng(12345)
    r = np.arange(N, dtype=np.int64)
    for attempt in range(4):
        for _ in range(200):
            rs, rd = r[src], r[dst]
            k = rd // C
            wg = k * WPC + (rd - k * C) // 128
            cl = rs % CLS
            cnt = np.bincount(wg * CLS + cl, minlength=WPC * P * CLS)
            cnt = cnt.reshape(WPC * P, CLS)
            over = np.argwhere(cnt > CAP)
            if len(over) == 0:
                return r
            # indeg per (new node, class)
            ind = np.zeros((N, CLS), np.int32)
            np.add.at(ind, (rd, cl), 1)
            inv = np.empty(N, np.int64)
            inv[r] = np.arange(N)
            # positions (new ids) grouped by window
            newpos = np.arange(N)
            kk = newpos // C
            wgpos = kk * WPC + (newpos - kk * C) // 128
            ok = True
            for wi, ci in over[:64]:
                excess = cnt[wi, ci] - CAP
                # nodes (new positions) in window wi, sorted by class-ci indeg desc
                pos_w = np.where(wgpos == wi)[0]
                nodes_w = inv[pos_w]
                order = np.argsort(-ind[pos_w, ci])
                # candidate receiving windows: class-ci count smallest
                recv = np.argsort(cnt[:, ci])[:32]
                moved = 0
                for oi in order:
                    if moved >= excess:
                        break
                    p_pos = pos_w[oi]
                    load = ind[p_pos, :]
                    done = False
                    for w2 in recv:
                        if w2 == wi:
                            continue
                        pos_w2 = np.where(wgpos == w2)[0]
                        pos_w2 = pos_w2[pos_w2 % CLS == p_pos % CLS]
                        if len(pos_w2) == 0:
                            continue
                        q_pos = pos_w2[np.argmin(ind[pos_w2, ci])]
                        load2 = ind[q_pos, :]
                        if np.all(cnt[w2] + load - load2 <= CAP) and np.all(
                            cnt[wi] - load + load2 <= cnt[wi]
                        ):
                            a, b = inv[p_pos], inv[q_pos]
                            r[a], r[b] = r[b], r[a]
                            done = True
                            break
                    if done:
                        moved += load[ci] - 0
                if not ok:
                    break
            # loop continues; recount next iteration
        # full restart with a random mod-4-preserving shuffle
        base = np.arange(N, dtype=np.int64)
        for m in range(CLS):
            cls_pos = base[m::CLS]
            rng.shuffle(cls_pos)
            base[m::CLS] = cls_pos
        r = base
    raise RuntimeError("node assignment repair failed")


def _preprocess(inputs):
    x = np.asarray(inputs["x"], np.float32)
    ei = np.asarray(inputs["edge_index"], np.int64)
    batch = np.asarray(inputs["batch"], np.int64)
    W1 = np.asarray(inputs["W1"], np.float32)
    b1 = np.asarray(inputs["b1"], np.float32)
    W2 = np.asarray(inputs["W2"], np.float32)
    b2 = np.asarray(inputs["b2"], np.float32)
    Wl = np.asarray(inputs["Wl"], np.float32)
    bl = np.asarray(inputs["bl"], np.float32)

    src = np.concatenate([ei[0], np.arange(N, dtype=np.int64)])
    dst = np.concatenate([ei[1], np.arange(N, dtype=np.int64)])
    deg = np.bincount(dst, minlength=N).astype(np.float32)
    dinv = 1.0 / np.sqrt(deg)
    sqdeg = np.sqrt(deg)

    r = _assign(src, dst)            # new id per original node
    inv = np.empty(N, np.int64)
    inv[r] = np.arange(N)            # original node per new id

    # per-(new) node arrays
    dinv_n = dinv[inv]
    sqdeg_n = sqdeg[inv]
    batch_n = batch[inv]
    table0 = (dinv[:, None] * x)[inv].astype(np.float16)  # [N, F] new order

    rs, rd = r[src], r[dst]
    k = rd // C
    shared = {
        "table0": table0,
        "iota128": np.tile(np.arange(128, dtype=np.float16), (128, 1)),
        "giota512": np.tile(np.arange(512, dtype=np.float16), (128, 1)),
        "ones_row": np.ones((1, 128), np.float32),
        "w1": W1.astype(np.float16),
        "w2": W2.astype(np.float16),
        "b1r": b1[None, :].astype(np.float16),
        "b2r": b2[None, :].astype(np.float16),
        "wl": Wl.astype(np.float32),
        "blr": bl[None, :].astype(np.float32),
    }

    per_core = []
    for kk in range(P):
        sel = k == kk
        ms, md = rs[sel], rd[sel] - kk * C   # md in [0, C)
        w = md // 128
        cl = ms % CLS
        li = ms // CLS                        # gather idx within class view
        dl = md - 128 * w                     # dst slot within window

        # slot layout: class stream c, window-major cells of CAP slots
        order = np.lexsort((md, w * CLS + cl))  # group by (w, c); md minor (any)
        ms_o, w_o, cl_o, li_o, dl_o = ms[order], w[order], cl[order], li[order], dl[order]
        cellcnt = np.bincount(w_o * CLS + cl_o, minlength=WPC * CLS).reshape(WPC, CLS)
        if cellcnt.max() > CAP:
            raise RuntimeError("cell overflow after assignment")

        gidx = np.zeros((CLS, CLSIDX), np.int16)          # pad idx 0
        dloc = np.full((CLS, CLSIDX), -1.0, np.float16)   # pad dstloc -1
        # place each (w, c) run at offset w*CAP in class stream c
        starts = np.zeros((WPC, CLS), np.int64)
        pos = 0
        for wi in range(WPC):
            for ci in range(CLS):
                n_ = cellcnt[wi, ci]
                seg = slice(pos, pos + n_)
                gidx[ci, wi * CAP : wi * CAP + n_] = li_o[seg].astype(np.int16)
                dloc[ci, wi * CAP : wi * CAP + n_] = dl_o[seg].astype(np.float16)
                pos += n_
        assert pos == ms.shape[0]

        # wrapped gather idx, class blocks concatenated: [128, CLS*CLSC]
        gw = np.concatenate([_wrap_idx(gidx[ci]) for ci in range(CLS)], axis=1)

        # dstloc columns ordered by (w, c, tile): [128, NTGT]
        dcols = np.empty((128, NTGT), np.float16)
        for wi in range(WPC):
            for ci in range(CLS):
                blk = dloc[ci, wi * CAP : (wi + 1) * CAP].reshape(TPC, 128).T
                dcols[:, (wi * TPW + ci * TPC) : (wi * TPW + (ci + 1) * TPC)] = blk

        def cols(vals, pad):
            v = np.full(CPAD, pad, vals.dtype)
            v[:C] = vals
            return v.reshape(WPC, 128).T.copy()

        per_core.append(
            {
                "gidx": gw,
                "dstloc": dcols,
                "dinv_c": cols(dinv_n[kk * C : (kk + 1) * C], np.float32(0)),
                "dinv2_c": cols((dinv_n * dinv_n)[kk * C : (kk + 1) * C], np.float32(0)),
                "sqdeg_r": np.concatenate(
                    [sqdeg_n[kk * C : (kk + 1) * C], np.zeros(CPAD - C)]
                ).astype(np.float16)[None, :],
                "bloc_c": cols(
                    batch_n[kk * C : (kk + 1) * C].astype(np.float16),
                    np.float16(-1),
                ),
            }
        )
    return shared, per_core


def _build_program():
    if "nc" in _cache:
        return _cache["nc"]
    import os
    scope = os.environ.get("GCN_SCOPE", "full")  # full | nocoll | l1 | gather
    import concourse.bacc as bacc
    import concourse.mybir as mybir
    import concourse.tile as tile
    from concourse.bass import AP

    f16 = mybir.dt.float16
    f32 = mybir.dt.float32
    i16 = mybir.dt.int16

    nc = bacc.Bacc("TRN2", target_bir_lowering=False, debug=False, num_devices=P)

    table0_d = nc.dram_tensor("table0", [N, F], f16, kind="ExternalInput")
    gidx_d = nc.dram_tensor("gidx", [128, CLS * CLSC], i16, kind="ExternalInput")
    dstloc_d = nc.dram_tensor("dstloc", [128, NTGT], f16, kind="ExternalInput")
    dinv_d = nc.dram_tensor("dinv_c", [128, WPC], f32, kind="ExternalInput")
    dinv2_d = nc.dram_tensor("dinv2_c", [128, WPC], f32, kind="ExternalInput")
    sqdeg_d = nc.dram_tensor("sqdeg_r", [1, CPAD], f16, kind="ExternalInput")
    bloc_d = nc.dram_tensor("bloc_c", [128, WPC], f16, kind="ExternalInput")
    iota_d = nc.dram_tensor("iota128", [128, 128], f16, kind="ExternalInput")
    giota_d = nc.dram_tensor("giota512", [128, 512], f16, kind="ExternalInput")
    ones_d = nc.dram_tensor("ones_row", [1, 128], f32, kind="ExternalInput")
    w1_d = nc.dram_tensor("w1", [F, F], f16, kind="ExternalInput")
    w2_d = nc.dram_tensor("w2", [F, F], f16, kind="ExternalInput")
    b1_d = nc.dram_tensor("b1r", [1, F], f16, kind="ExternalInput")
    b2_d = nc.dram_tensor("b2r", [1, F], f16, kind="ExternalInput")
    wl_d = nc.dram_tensor("wl", [F, OUT], f32, kind="ExternalInput")
    bl_d = nc.dram_tensor("blr", [1, OUT], f32, kind="ExternalInput")

    t1loc = nc.dram_tensor("t1loc", [C, F], f16)
    if os.environ.get("GCN_T1_SHARED", "1") == "1":
        t1full = nc.dram_tensor("t1full", [N, F], f16, addr_space="Shared")
    else:
        t1full = nc.dram_tensor("t1full", [N, F], f16)
    poolb = nc.dram_tensor("poolb", [128, G], f32)
    poolr = nc.dram_tensor("poolr", [128, G], f32, addr_space="Shared")
    out_d = nc.dram_tensor("out", [G, OUT], f32, kind="ExternalOutput")

    relu = mybir.ActivationFunctionType.Relu
    iseq = mybir.AluOpType.is_equal

    with tile.TileContext(nc) as tc:
        with (
            tc.tile_pool(name="const", bufs=1) as cst,
            tc.tile_pool(name="idx", bufs=2) as idxp,
            tc.tile_pool(name="msg", bufs=2) as msgp,
            tc.tile_pool(name="oh", bufs=3) as ohp,
            tc.tile_pool(name="small", bufs=3) as smp,
            tc.tile_pool(name="aggps", bufs=2, space="PSUM") as aggp,
            tc.tile_pool(name="trps", bufs=2, space="PSUM") as trp,
            tc.tile_pool(name="poolps", bufs=1, space="PSUM") as plp,
        ):
            def load_const(name, dram, shape, dt):
                t = cst.tile(shape, dt, tag=name)
                nc.sync.dma_start(out=t[:], in_=dram[:])
                return t

            iota_t = load_const("iota", iota_d, [128, 128], f16)
            giota_t = load_const("giota", giota_d, [128, 512], f16)
            dstloc_t = load_const("dstloc", dstloc_d, [128, NTGT], f16)
            dinv_t = load_const("dinv", dinv_d, [128, WPC], f32)
            dinv2_t = load_const("dinv2", dinv2_d, [128, WPC], f32)
            sqdeg_t = load_const("sqdeg", sqdeg_d, [1, CPAD], f16)
            bloc_t = load_const("bloc", bloc_d, [128, WPC], f16)
            ones_t = load_const("ones", ones_d, [1, 128], f32)
            w1_t = load_const("w1", w1_d, [F, F], f16)
            w2_t = load_const("w2", w2_d, [F, F], f16)
            b1_t = load_const("b1", b1_d, [1, F], f16)
            b2_t = load_const("b2", b2_d, [1, F], f16)
            wl_t = load_const("wl", wl_d, [F, OUT], f32)
            bl_t = load_const("bl", bl_d, [1, OUT], f32)

            n_repeat = int(os.environ.get("GCN_REPEAT", "1"))
            n_layers = 1 if scope in ("l1", "gather") else 2
            for _rep in range(n_repeat):
              pool_ps = plp.tile([128, G], f32, tag="poolps")
              for L in range(n_layers):
                table = table0_d if L == 0 else t1full
                Wt = w1_t if L == 0 else w2_t
                bt = b1_t if L == 0 else b2_t
                scale_t = dinv2_t if L == 0 else dinv_t

                for s in range(NSEG):
                    msgs = []
                    for ci in range(CLS):
                        it = idxp.tile([128, SEGC], i16, tag=f"idx{ci}")
                        nc.sync.dma_start(
                            out=it[:],
                            in_=gidx_d[:, ci * CLSC + s * SEGC : ci * CLSC + (s + 1) * SEGC],
                        )
                        mt = msgp.tile([128, SEGT, F], f16, tag=f"msg{ci}")
                        view = AP(table, ci * F, [[CLS * F, NPC], [1, F]])
                        nc.gpsimd.dma_gather(
                            mt[:], view, it[:], SEGIDX, SEGIDX, F,
                            elem_step=CLS * F, single_packet=False,
                        )
                        msgs.append(mt)

                    if scope == "gather":
                        sink = smp.tile([128, 128], f16, tag="sink")
                        nc.vector.tensor_copy(out=sink[:], in_=msgs[0][:, 0, :])
                        continue

                    for wl_ in range(SEGW):
                        w = s * SEGW + wl_
                        oh = ohp.tile([128, TPW, 128], f16, tag="oh")
                        in0 = dstloc_t[:, w * TPW : (w + 1) * TPW].to_broadcast(
                            [128, TPW, 128]
                        )
                        i_ap = iota_t[:]
                        in1 = AP(
                            i_ap.tensor,
                            i_ap.offset,
                            [list(i_ap.ap[0]), [0, TPW], [1, 128]],
                        )
                        nc.vector.tensor_tensor(out=oh[:], in0=in0, in1=in1, op=iseq)

                        agg = aggp.tile([128, 128], f32, tag="agg")
                        for ci in range(CLS):
                            for ti in range(TPC):
                                nc.tensor.matmul(
                                    out=agg[:],
                                    lhsT=msgs[ci][:, wl_ * TPC + ti, :],
                                    rhs=oh[:, ci * TPC + ti, :],
                                    start=(ci == 0 and ti == 0),
                                    stop=(ci == CLS - 1 and ti == TPC - 1),
                                )
                        aggsb = smp.tile([128, 128], f16, tag="aggsb")
                        nc.scalar.copy(out=aggsb[:], in_=agg[:])

                        tp = trp.tile([128, 128], f32, tag="tp")
                        nc.tensor.matmul(
                            out=tp[:], lhsT=aggsb[:], rhs=Wt[:], start=True, stop=False
                        )
                        nc.tensor.matmul(
                            out=tp[:],
                            lhsT=sqdeg_t[0:1, w * 128 : (w + 1) * 128],
                            rhs=bt[:],
                            start=False,
                            stop=True,
                        )
                        tab = smp.tile([128, 128], f16, tag="tab")
                        nc.scalar.activation(
                            out=tab[:], in_=tp[:], func=relu,
                            scale=scale_t[:, w : w + 1],
                        )
                        if L == 0:
                            rows = LASTW if w == WPC - 1 else 128
                            nc.sync.dma_start(
                                out=t1loc[w * 128 : w * 128 + rows, :],
                                in_=tab[0:rows, :],
                            )
                        else:
                            ohb = ohp.tile([128, G], f16, tag="ohb")
                            nc.vector.tensor_tensor(
                                out=ohb[:],
                                in0=bloc_t[:, w : w + 1].to_broadcast([128, G]),
                                in1=giota_t[:],
                                op=iseq,
                            )
                            nc.tensor.matmul(
                                out=pool_ps[:],
                                lhsT=tab[:],
                                rhs=ohb[:],
                                start=(w == 0),
                                stop=(w == WPC - 1),
                            )
                if L == 0 and n_layers == 2:
                    if scope == "full":
                        nc.gpsimd.collective_compute(
                            "AllGather",
                            mybir.AluOpType.bypass,
                            replica_groups=[list(range(P))],
                            ins=[t1loc[:]],
                            outs=[t1full[:]],
                        )
                    else:
                        nc.gpsimd.dma_start(out=t1full[0:C, :], in_=t1loc[:])

            if scope in ("l1", "gather"):
                zt = smp.tile([128, OUT], f32, tag="zt")
                nc.vector.memset(zt[:], 0.0)
                for gs in range(G // 128):
                    nc.sync.dma_start(
                        out=out_d[gs * 128 : (gs + 1) * 128, :], in_=zt[:]
                    )
            else:
                poolsb = smp.tile([128, G], f32, tag="poolsb")
                nc.scalar.copy(out=poolsb[:], in_=pool_ps[:])
                nc.gpsimd.dma_start(out=poolb[:], in_=poolsb[:])
                if scope == "full":
                    nc.gpsimd.collective_compute(
                        "AllReduce",
                        mybir.AluOpType.add,
                        replica_groups=[list(range(P))],
                        ins=[poolb[:]],
                        outs=[poolr[:]],
                    )
                else:
                    nc.gpsimd.dma_start(out=poolr[:], in_=poolb[:])
                prsb = smp.tile([128, G], f32, tag="prsb")
                nc.sync.dma_start(out=prsb[:], in_=poolr[:])
                for gs in range(G // 128):
                    fps = trp.tile([128, OUT], f32, tag="fps")
                    nc.tensor.matmul(
                        out=fps[:],
                        lhsT=prsb[:, gs * 128 : (gs + 1) * 128],
                        rhs=wl_t[:],
                        start=True,
                        stop=False,
                    )
                    nc.tensor.matmul(
                        out=fps[:], lhsT=ones_t[0:1, :], rhs=bl_t[:],
                        start=False, stop=True,
                    )
                    osb = smp.tile([128, OUT], f32, tag="osb")
                    nc.scalar.copy(out=osb[:], in_=fps[:])
                    nc.sync.dma_start(
                        out=out_d[gs * 128 : (gs + 1) * 128, :], in_=osb[:]
                    )

    nc.compile()
    _cache["nc"] = nc
    return nc


def kernel(**inputs):
    from concourse.bass_utils import run_bass_kernel_spmd

    shared, per_core = _preprocess(inputs)
    nc = _build_program()
    in_maps = [{**shared, **pc} for pc in per_core]
    res = run_bass_kernel_spmd(nc, in_maps, list(range(P))).results
    return res[0]["out"].astype(np.float32)


# revision 20
# speedup vs baseline: 1.2950x; 1.2950x over previous
"""GCN (2x GCNConv + global_add_pool + Linear) on 8 Trainium2 NeuronCores.

Strategy (edge-cut data parallel, hardcoded for N=100000, E=1600000, F=128,
OUT=64, G=512, 8 cores):

  * Symmetric normalization factorizes: norm = dinv[src]*dinv[dst], so we
    pre-scale the gather table by dinv and post-scale the aggregate by dinv.
  * The linear transform commutes with aggregation:
    segsum(x[src]) @ W == segsum((x@W)[src]), so each layer is
    SpMM(gather+segsum) -> small local matmul -> bias -> relu.
  * Nodes are partitioned contiguously across the 8 cores (12500 each);
    every message (edge or self-loop) is processed by the core owning its
    dst. Messages are gathered from a replicated fp16 node table in DRAM
    with dma_gather (int16 indices => 4 stride-4 "classes" of table rows).
  * Aggregation happens on-chip: messages land in SBUF tiles of 128, a
    one-hot [msg x dst-slot] fp16 matrix is built on VectorE (is_equal vs
    an iota), and TensorE accumulates agg^T[f, dst] in PSUM per 128-node
    window. Cells (window x class) have a static capacity of 640 slots
    (5 tiles); pads gather row 0 and carry dstloc=-1 (no one-hot match).
  * Layer boundary: each core computes its 12500-row slice of the next
    pre-scaled table; AllGather replicates it for the next layer's gather.
  * Pooling: per window, one-hot [node x graph] matmul accumulates
    pooled^T[f, g] in PSUM; AllReduce over cores; final Linear is computed
    redundantly on every core.
"""

import numpy as np

N = 100000
E = 1600000
F = 128
OUT = 64
G = 512
P = 8
C = N // P            # 12500 nodes per core
CLS = 4               # table row classes (stride trick for int16 gather idx)
NPC = N // CLS        # rows per class view
WPC = 98              # windows per core (ceil(12500/128))
CPAD = WPC * 128      # 12544 padded nodes per core
LASTW = C - (WPC - 1) * 128  # 84 real nodes in the last window
import os as _os
CAP = int(_os.environ.get("GCN_CAP", "576"))  # slots per (window, class) cell
TGTW = 5              # matmul targets per (window, class); holds for CAP 576/640
TPW = CLS * TGTW      # targets per window
_SEGW = {576: 8, 640: 7}[CAP]   # windows per gather segment (tile-aligned)
_nfull = WPC // _SEGW
SEGWS = [_SEGW] * _nfull + ([WPC - _SEGW * _nfull] if WPC % _SEGW else [])
SEGOF = [0] + list(np.cumsum(SEGWS).tolist())  # window offset per segment
NSEG = len(SEGWS)
CLSIDX = WPC * CAP    # 56448 idxs per class stream
CLSC = CLSIDX // 16   # 3528 idx columns per class stream
NTGT = WPC * TPW      # 1960 one-hot target columns per layer


def _tiles_of_window(w):
    """Stream tiles touched by cell of window w (within its class stream)."""
    t0 = (w * CAP) // 128
    t1 = (w * CAP + CAP - 1) // 128
    return list(range(t0, t1 + 1))

_cache = {}


def _wrap_idx(idx):
    """[n] -> [128, n//16] int16, wrapped in 16 partitions, replicated x8."""
    n = idx.shape[0]
    w = idx.reshape(n // 16, 16).T.astype(np.int16)
    return np.tile(w, (P, 1))


def _assign(src, dst):
    """Choose node relabeling r (new id per node) s.t. every
    (core-window, class) cell count <= CAP, via class-preserving swaps
    (swapped nodes keep their id mod CLS, so all message classes are
    invariant and only window membership changes)."""
    rng = np.random.default_rng(0)
    for attempt in range(3):
        r = np.arange(N, dtype=np.int64)
        if attempt > 0:
            for m in range(CLS):
                pos = r[m::CLS].copy()
                rng.shuffle(pos)
                r[m::CLS] = pos
        cls_of_src = r[src] % CLS
        ind = np.zeros((N, CLS), np.int32)
        np.add.at(ind, (dst, cls_of_src), 1)
        rk = r // C
        wg = rk * WPC + (r - rk * C) // 128
        cnt = np.zeros((WPC * P, CLS), np.int64)
        for c in range(CLS):
            np.add.at(cnt[:, c], wg, ind[:, c])
        members = [[] for _ in range(WPC * P)]
        for n in range(N):
            members[wg[n]].append(n)
        members = [np.array(m) for m in members]
        ok = True
        for _ in range(100000):
            over = np.argwhere(cnt > CAP)
            if len(over) == 0:
                return r
            wi, ci = over[rng.integers(len(over))]
            mem = members[wi]
            n = mem[np.argmax(ind[mem, ci])]
            head = CAP - cnt[:, ci] - ind[n, ci]
            cand = np.argsort(-head)[:64]
            done = False
            for w2 in cand:
                if w2 == wi or head[w2] < 0:
                    continue
                mem2 = members[w2]
                mem2c = mem2[(r[mem2] % CLS) == (r[n] % CLS)]
                if len(mem2c) == 0:
                    continue
                m = mem2c[np.argmin(ind[mem2c, ci])]
                new_w2 = cnt[w2] + ind[n] - ind[m]
                new_wi = cnt[wi] - ind[n] + ind[m]
                if np.all(new_w2 <= CAP) and new_wi[ci] < cnt[wi, ci]:
                    r[n], r[m] = r[m], r[n]
                    cnt[w2], cnt[wi] = new_w2, new_wi
                    wg[n], wg[m] = w2, wi
                    members[wi] = np.append(mem[mem != n], m)
                    members[w2] = np.append(mem2[mem2 != m], n)
                    done = True
                    break
            if not done:
                ok = False
                break
        if ok:
            continue
    raise RuntimeError("node assignment repair failed")


def _preprocess(inputs):
    x = np.asarray(inputs["x"], np.float32)
    ei = np.asarray(inputs["edge_index"], np.int64)
    batch = np.asarray(inputs["batch"], np.int64)
    W1 = np.asarray(inputs["W1"], np.float32)
    b1 = np.asarray(inputs["b1"], np.float32)
    W2 = np.asarray(inputs["W2"], np.float32)
    b2 = np.asarray(inputs["b2"], np.float32)
    Wl = np.asarray(inputs["Wl"], np.float32)
    bl = np.asarray(inputs["bl"], np.float32)

    src = np.concatenate([ei[0], np.arange(N, dtype=np.int64)])
    dst = np.concatenate([ei[1], np.arange(N, dtype=np.int64)])
    deg = np.bincount(dst, minlength=N).astype(np.float32)
    dinv = 1.0 / np.sqrt(deg)
    sqdeg = np.sqrt(deg)

    r = _assign(src, dst)            # new id per original node
    inv = np.empty(N, np.int64)
    inv[r] = np.arange(N)            # original node per new id

    # per-(new) node arrays
    dinv_n = dinv[inv]
    sqdeg_n = sqdeg[inv]
    batch_n = batch[inv]
    table0 = (dinv[:, None] * x)[inv].astype(np.float16)  # [N, F] new order

    rs, rd = r[src], r[dst]
    k = rd // C
    shared = {
        "table0": table0,
        "iota128": np.tile(np.arange(128, dtype=np.float16), (128, 1)),
        "giota512": np.tile(np.arange(512, dtype=np.float16), (128, 1)),
        "ones_row": np.ones((1, 128), np.float32),
        "w1": W1.astype(np.float16),
        "w2": W2.astype(np.float16),
        "b1r": b1[None, :].astype(np.float16),
        "b2r": b2[None, :].astype(np.float16),
        "wl": Wl.astype(np.float32),
        "blr": bl[None, :].astype(np.float32),
    }

    per_core = []
    for kk in range(P):
        sel = k == kk
        ms, md = rs[sel], rd[sel] - kk * C   # md in [0, C)
        w = md // 128
        cl = ms % CLS
        li = ms // CLS                        # gather idx within class view
        dl = md - 128 * w                     # dst slot within window

        # slot layout: class stream c, window-major cells of CAP slots
        order = np.lexsort((md, w * CLS + cl))  # group by (w, c); md minor (any)
        w_o, cl_o, li_o, md_o = w[order], cl[order], li[order], md[order]
        cellcnt = np.bincount(w_o * CLS + cl_o, minlength=WPC * CLS).reshape(WPC, CLS)
        if cellcnt.max() > CAP:
            raise RuntimeError("cell overflow after assignment")

        gidx = np.zeros((CLS, CLSIDX), np.int16)            # pad idx 0
        dabs = np.full((CLS, CLSIDX), -1, np.int64)         # absolute dst_local; pad -1
        # place each (w, c) run at offset w*CAP in class stream c
        pos = 0
        for wi in range(WPC):
            for ci in range(CLS):
                n_ = cellcnt[wi, ci]
                seg = slice(pos, pos + n_)
                gidx[ci, wi * CAP : wi * CAP + n_] = li_o[seg].astype(np.int16)
                dabs[ci, wi * CAP : wi * CAP + n_] = md_o[seg]
                pos += n_
        assert pos == ms.shape[0]

        # wrapped gather idx, class blocks concatenated: [128, CLS*CLSC]
        gw = np.concatenate([_wrap_idx(gidx[ci]) for ci in range(CLS)], axis=1)

        # dstloc columns ordered by (w, c, target-tile): [128, NTGT]
        # value = dst_local - 128*w (window-relative); pads and slots of other
        # cells fall outside [0, 128) and never match the iota.
        dcols = np.empty((128, NTGT), np.float16)
        for wi in range(WPC):
            tiles = _tiles_of_window(wi)
            for ci in range(CLS):
                for ti, t in enumerate(tiles):
                    sl = dabs[ci, t * 128 : (t + 1) * 128]
                    rel = np.where(sl < 0, -1, sl - 128 * wi)
                    dcols[:, wi * TPW + ci * TGTW + ti] = rel.astype(np.float16)

        def cols(vals, pad):
            v = np.full(CPAD, pad, vals.dtype)
            v[:C] = vals
            return v.reshape(WPC, 128).T.copy()

        per_core.append(
            {
                "gidx": gw,
                "dstloc": dcols,
                "dinv_c": cols(dinv_n[kk * C : (kk + 1) * C], np.float32(0)),
                "dinv2_c": cols((dinv_n * dinv_n)[kk * C : (kk + 1) * C], np.float32(0)),
                "sqdeg_r": np.concatenate(
                    [sqdeg_n[kk * C : (kk + 1) * C], np.zeros(CPAD - C)]
                ).astype(np.float16)[None, :],
                "bloc_c": cols(
                    batch_n[kk * C : (kk + 1) * C].astype(np.float16),
                    np.float16(-1),
                ),
            }
        )
    return shared, per_core


def _build_program():
    if "nc" in _cache:
        return _cache["nc"]
    import os
    scope = os.environ.get("GCN_SCOPE", "full")  # full | nocoll | l1 | gather
    import concourse.bacc as bacc
    import concourse.mybir as mybir
    import concourse.tile as tile
    from concourse.bass import AP

    f16 = mybir.dt.float16
    f32 = mybir.dt.float32
    i16 = mybir.dt.int16

    nc = bacc.Bacc("TRN2", target_bir_lowering=False, debug=False, num_devices=P)

    table0_d = nc.dram_tensor("table0", [N, F], f16, kind="ExternalInput")
    gidx_d = nc.dram_tensor("gidx", [128, CLS * CLSC], i16, kind="ExternalInput")
    dstloc_d = nc.dram_tensor("dstloc", [128, NTGT], f16, kind="ExternalInput")
    dinv_d = nc.dram_tensor("dinv_c", [128, WPC], f32, kind="ExternalInput")
    dinv2_d = nc.dram_tensor("dinv2_c", [128, WPC], f32, kind="ExternalInput")
    sqdeg_d = nc.dram_tensor("sqdeg_r", [1, CPAD], f16, kind="ExternalInput")
    bloc_d = nc.dram_tensor("bloc_c", [128, WPC], f16, kind="ExternalInput")
    iota_d = nc.dram_tensor("iota128", [128, 128], f16, kind="ExternalInput")
    giota_d = nc.dram_tensor("giota512", [128, 512], f16, kind="ExternalInput")
    ones_d = nc.dram_tensor("ones_row", [1, 128], f32, kind="ExternalInput")
    w1_d = nc.dram_tensor("w1", [F, F], f16, kind="ExternalInput")
    w2_d = nc.dram_tensor("w2", [F, F], f16, kind="ExternalInput")
    b1_d = nc.dram_tensor("b1r", [1, F], f16, kind="ExternalInput")
    b2_d = nc.dram_tensor("b2r", [1, F], f16, kind="ExternalInput")
    wl_d = nc.dram_tensor("wl", [F, OUT], f32, kind="ExternalInput")
    bl_d = nc.dram_tensor("blr", [1, OUT], f32, kind="ExternalInput")

    t1loc = nc.dram_tensor("t1loc", [C, F], f16)
    # NOTE: t1full must be ordinary DRAM. addr_space="Shared" works for the
    # AllGather but makes the random-access gather reads ~8x slower.
    t1full = nc.dram_tensor("t1full", [N, F], f16)
    poolb = nc.dram_tensor("poolb", [128, G], f32)
    poolr = nc.dram_tensor("poolr", [128, G], f32, addr_space="Shared")
    out_d = nc.dram_tensor("out", [G, OUT], f32, kind="ExternalOutput")

    relu = mybir.ActivationFunctionType.Relu
    iseq = mybir.AluOpType.is_equal

    with tile.TileContext(nc) as tc:
        with (
            tc.tile_pool(name="const", bufs=1) as cst,
            tc.tile_pool(name="idx", bufs=2) as idxp,
            tc.tile_pool(name="msg", bufs=2) as msgp,
            tc.tile_pool(name="oh", bufs=3) as ohp,
            tc.tile_pool(name="small", bufs=3) as smp,
            tc.tile_pool(name="aggps", bufs=2, space="PSUM") as aggp,
            tc.tile_pool(name="trps", bufs=2, space="PSUM") as trp,
            tc.tile_pool(name="poolps", bufs=1, space="PSUM") as plp,
        ):
            def load_const(name, dram, shape, dt):
                t = cst.tile(shape, dt, tag=name)
                nc.sync.dma_start(out=t[:], in_=dram[:])
                return t

            iota_t = load_const("iota", iota_d, [128, 128], f16)
            giota_t = load_const("giota", giota_d, [128, 512], f16)
            dstloc_t = load_const("dstloc", dstloc_d, [128, NTGT], f16)
            dinv_t = load_const("dinv", dinv_d, [128, WPC], f32)
            dinv2_t = load_const("dinv2", dinv2_d, [128, WPC], f32)
            sqdeg_t = load_const("sqdeg", sqdeg_d, [1, CPAD], f16)
            bloc_t = load_const("bloc", bloc_d, [128, WPC], f16)
            ones_t = load_const("ones", ones_d, [1, 128], f32)
            w1_t = load_const("w1", w1_d, [F, F], f16)
            w2_t = load_const("w2", w2_d, [F, F], f16)
            b1_t = load_const("b1", b1_d, [1, F], f16)
            b2_t = load_const("b2", b2_d, [1, F], f16)
            wl_t = load_const("wl", wl_d, [F, OUT], f32)
            bl_t = load_const("bl", bl_d, [1, OUT], f32)

            n_repeat = int(os.environ.get("GCN_REPEAT", "1"))
            n_layers = 1 if scope in ("l1", "gather") else 2
            for _rep in range(n_repeat):
              pool_ps = plp.tile([128, G], f32, tag="poolps")
              for L in range(n_layers):
                table = table0_d if L == 0 else t1full
                Wt = w1_t if L == 0 else w2_t
                bt = b1_t if L == 0 else b2_t
                scale_t = dinv2_t if L == 0 else dinv_t

                for s in range(NSEG):
                    nwin = SEGWS[s]
                    wb = SEGOF[s]
                    nidx = nwin * CAP
                    segt = nidx // 128
                    segc = nidx // 16
                    tbase = wb * CAP // 128
                    msgs = []
                    for ci in range(CLS):
                        it = idxp.tile([128, segc], i16, tag=f"idx{ci}")
                        cb = ci * CLSC + wb * CAP // 16
                        nc.sync.dma_start(
                            out=it[:], in_=gidx_d[:, cb : cb + segc]
                        )
                        mt = msgp.tile([128, segt, F], f16, tag=f"msg{ci}")
                        view = AP(table, ci * F, [[CLS * F, NPC], [1, F]])
                        nc.gpsimd.dma_gather(
                            mt[:], view, it[:], nidx, nidx, F,
                            elem_step=CLS * F, single_packet=False,
                        )
                        msgs.append(mt)

                    if scope == "gather":
                        sink = smp.tile([128, 128], f16, tag="sink")
                        nc.vector.tensor_copy(out=sink[:], in_=msgs[0][:, 0, :])
                        continue

                    for wl_ in range(nwin):
                        w = wb + wl_
                        oh = ohp.tile([128, TPW, 128], f16, tag="oh")
                        in0 = dstloc_t[:, w * TPW : (w + 1) * TPW].to_broadcast(
                            [128, TPW, 128]
                        )
                        i_ap = iota_t[:]
                        in1 = AP(
                            i_ap.tensor,
                            i_ap.offset,
                            [list(i_ap.ap[0]), [0, TPW], [1, 128]],
                        )
                        nc.vector.tensor_tensor(out=oh[:], in0=in0, in1=in1, op=iseq)

                        agg = aggp.tile([128, 128], f32, tag="agg")
                        wtiles = _tiles_of_window(w)
                        for ci in range(CLS):
                            for ti, t in enumerate(wtiles):
                                nc.tensor.matmul(
                                    out=agg[:],
                                    lhsT=msgs[ci][:, t - tbase, :],
                                    rhs=oh[:, ci * TGTW + ti, :],
                                    start=(ci == 0 and ti == 0),
                                    stop=(ci == CLS - 1 and ti == TGTW - 1),
                                )
                        aggsb = smp.tile([128, 128], f16, tag="aggsb")
                        nc.scalar.copy(out=aggsb[:], in_=agg[:])

                        tp = trp.tile([128, 128], f32, tag="tp")
                        nc.tensor.matmul(
                            out=tp[:], lhsT=aggsb[:], rhs=Wt[:], start=True, stop=False
                        )
                        nc.tensor.matmul(
                            out=tp[:],
                            lhsT=sqdeg_t[0:1, w * 128 : (w + 1) * 128],
                            rhs=bt[:],
                            start=False,
                            stop=True,
                        )
                        tab = smp.tile([128, 128], f16, tag="tab")
                        nc.scalar.activation(
                            out=tab[:], in_=tp[:], func=relu,
                            scale=scale_t[:, w : w + 1],
                        )
                        if L == 0:
                            rows = LASTW if w == WPC - 1 else 128
                            nc.sync.dma_start(
                                out=t1loc[w * 128 : w * 128 + rows, :],
                                in_=tab[0:rows, :],
                            )
                        else:
                            ohb = ohp.tile([128, G], f16, tag="ohb")
                            nc.vector.tensor_tensor(
                                out=ohb[:],
                                in0=bloc_t[:, w : w + 1].to_broadcast([128, G]),
                                in1=giota_t[:],
                                op=iseq,
                            )
                            nc.tensor.matmul(
                                out=pool_ps[:],
                                lhsT=tab[:],
                                rhs=ohb[:],
                                start=(w == 0),
                                stop=(w == WPC - 1),
                            )
                if L == 0 and n_layers == 2:
                    if scope == "full":
                        nc.gpsimd.collective_compute(
                            "AllGather",
                            mybir.AluOpType.bypass,
                            replica_groups=[list(range(P))],
                            ins=[t1loc[:]],
                            outs=[t1full[:]],
                        )
                    else:
                        nc.gpsimd.dma_start(out=t1full[0:C, :], in_=t1loc[:])

            if scope in ("l1", "gather"):
                zt = smp.tile([128, OUT], f32, tag="zt")
                nc.vector.memset(zt[:], 0.0)
                for gs in range(G // 128):
                    nc.sync.dma_start(
                        out=out_d[gs * 128 : (gs + 1) * 128, :], in_=zt[:]
                    )
            else:
                poolsb = smp.tile([128, G], f32, tag="poolsb")
                nc.scalar.copy(out=poolsb[:], in_=pool_ps[:])
                nc.gpsimd.dma_start(out=poolb[:], in_=poolsb[:])
                if scope == "full":
                    nc.gpsimd.collective_compute(
                        "AllReduce",
                        mybir.AluOpType.add,
                        replica_groups=[list(range(P))],
                        ins=[poolb[:]],
                        outs=[poolr[:]],
                    )
                else:
                    nc.gpsimd.dma_start(out=poolr[:], in_=poolb[:])
                prsb = smp.tile([128, G], f32, tag="prsb")
                nc.sync.dma_start(out=prsb[:], in_=poolr[:])
                for gs in range(G // 128):
                    fps = trp.tile([128, OUT], f32, tag="fps")
                    nc.tensor.matmul(
                        out=fps[:],
                        lhsT=prsb[:, gs * 128 : (gs + 1) * 128],
                        rhs=wl_t[:],
                        start=True,
                        stop=False,
                    )
                    nc.tensor.matmul(
                        out=fps[:], lhsT=ones_t[0:1, :], rhs=bl_t[:],
                        start=False, stop=True,
                    )
                    osb = smp.tile([128, OUT], f32, tag="osb")
                    nc.scalar.copy(out=osb[:], in_=fps[:])
                    nc.sync.dma_start(
                        out=out_d[gs * 128 : (gs + 1) * 128, :], in_=osb[:]
                    )

    nc.compile()
    _cache["nc"] = nc
    return nc


def kernel(**inputs):
    from concourse.bass_utils import run_bass_kernel_spmd

    shared, per_core = _preprocess(inputs)
    nc = _build_program()
    in_maps = [{**shared, **pc} for pc in per_core]
    res = run_bass_kernel_spmd(nc, in_maps, list(range(P))).results
    return res[0]["out"].astype(np.float32)


# revision 27
# speedup vs baseline: 4.1585x; 3.2113x over previous
"""GCN (2x GCNConv + global_add_pool + Linear) on 8 Trainium2 NeuronCores.

Strategy (edge-cut data parallel, hardcoded for N=100000, E=1600000, F=128,
OUT=64, G=512, 8 cores):

  * Symmetric normalization factorizes: norm = dinv[src]*dinv[dst], so we
    pre-scale the gather table by dinv and post-scale the aggregate by dinv.
  * The linear transform commutes with aggregation:
    segsum(x[src]) @ W == segsum((x@W)[src]), so each layer is
    SpMM(gather+segsum) -> small local matmul -> bias -> relu.
  * Nodes are partitioned contiguously across the 8 cores (12500 each);
    every message (edge or self-loop) is processed by the core owning its
    dst. Messages are gathered from a replicated fp16 node table in DRAM
    with dma_gather (int16 indices => 4 stride-4 "classes" of table rows).
  * Aggregation happens on-chip: messages land in SBUF tiles of 128, a
    one-hot [msg x dst-slot] fp16 matrix is built on VectorE (is_equal vs
    an iota), and TensorE accumulates agg^T[f, dst] in PSUM per 128-node
    window. Cells (window x class) have a static capacity of 640 slots
    (5 tiles); pads gather row 0 and carry dstloc=-1 (no one-hot match).
  * Layer boundary: each core computes its 12500-row slice of the next
    pre-scaled table; AllGather replicates it for the next layer's gather.
  * Pooling: per window, one-hot [node x graph] matmul accumulates
    pooled^T[f, g] in PSUM; AllReduce over cores; final Linear is computed
    redundantly on every core.
"""

import numpy as np

N = 100000
E = 1600000
F = 128
OUT = 64
G = 512
P = 8
C = N // P            # 12500 nodes per core
CLS = 4               # table row classes (stride trick for int16 gather idx)
NPC = N // CLS        # rows per class view
WPC = 98              # windows per core (ceil(12500/128))
CPAD = WPC * 128      # 12544 padded nodes per core
LASTW = C - (WPC - 1) * 128  # 84 real nodes in the last window
import os as _os
CAP = int(_os.environ.get("GCN_CAP", "576"))  # slots per (window, class) cell
TGTW = 5              # matmul targets per (window, class); holds for CAP 576/640
TPW = CLS * TGTW      # targets per window
_SEGW = {576: 8, 640: 7}[CAP]   # windows per gather segment (tile-aligned)
_nfull = WPC // _SEGW
SEGWS = [_SEGW] * _nfull + ([WPC - _SEGW * _nfull] if WPC % _SEGW else [])
SEGOF = [0] + list(np.cumsum(SEGWS).tolist())  # window offset per segment
NSEG = len(SEGWS)
CLSIDX = WPC * CAP    # 56448 idxs per class stream
CLSC = CLSIDX // 16   # 3528 idx columns per class stream
NTGT = WPC * TPW      # 1960 one-hot target columns per layer


def _tiles_of_window(w):
    """Stream tiles touched by cell of window w (within its class stream)."""
    t0 = (w * CAP) // 128
    t1 = (w * CAP + CAP - 1) // 128
    return list(range(t0, t1 + 1))

_cache = {}


def _wrap_idx(idx):
    """[n] -> [128, n//16] int16, wrapped in 16 partitions, replicated x8."""
    n = idx.shape[0]
    w = idx.reshape(n // 16, 16).T.astype(np.int16)
    return np.tile(w, (P, 1))


def _assign(src, dst):
    """Choose node relabeling r (new id per node) s.t. every
    (core-window, class) cell count <= CAP, via class-preserving swaps
    (swapped nodes keep their id mod CLS, so all message classes are
    invariant and only window membership changes)."""
    rng = np.random.default_rng(0)
    for attempt in range(3):
        r = np.arange(N, dtype=np.int64)
        if attempt > 0:
            for m in range(CLS):
                pos = r[m::CLS].copy()
                rng.shuffle(pos)
                r[m::CLS] = pos
        cls_of_src = r[src] % CLS
        ind = np.zeros((N, CLS), np.int32)
        np.add.at(ind, (dst, cls_of_src), 1)
        rk = r // C
        wg = rk * WPC + (r - rk * C) // 128
        cnt = np.zeros((WPC * P, CLS), np.int64)
        for c in range(CLS):
            np.add.at(cnt[:, c], wg, ind[:, c])
        members = [[] for _ in range(WPC * P)]
        for n in range(N):
            members[wg[n]].append(n)
        members = [np.array(m) for m in members]
        ok = True
        for _ in range(100000):
            over = np.argwhere(cnt > CAP)
            if len(over) == 0:
                return r
            wi, ci = over[rng.integers(len(over))]
            mem = members[wi]
            n = mem[np.argmax(ind[mem, ci])]
            head = CAP - cnt[:, ci] - ind[n, ci]
            cand = np.argsort(-head)[:64]
            done = False
            for w2 in cand:
                if w2 == wi or head[w2] < 0:
                    continue
                mem2 = members[w2]
                mem2c = mem2[(r[mem2] % CLS) == (r[n] % CLS)]
                if len(mem2c) == 0:
                    continue
                m = mem2c[np.argmin(ind[mem2c, ci])]
                new_w2 = cnt[w2] + ind[n] - ind[m]
                new_wi = cnt[wi] - ind[n] + ind[m]
                if np.all(new_w2 <= CAP) and new_wi[ci] < cnt[wi, ci]:
                    r[n], r[m] = r[m], r[n]
                    cnt[w2], cnt[wi] = new_w2, new_wi
                    wg[n], wg[m] = w2, wi
                    members[wi] = np.append(mem[mem != n], m)
                    members[w2] = np.append(mem2[mem2 != m], n)
                    done = True
                    break
            if not done:
                ok = False
                break
        if ok:
            continue
    raise RuntimeError("node assignment repair failed")


def _preprocess(inputs):
    x = np.asarray(inputs["x"], np.float32)
    ei = np.asarray(inputs["edge_index"], np.int64)
    batch = np.asarray(inputs["batch"], np.int64)
    W1 = np.asarray(inputs["W1"], np.float32)
    b1 = np.asarray(inputs["b1"], np.float32)
    W2 = np.asarray(inputs["W2"], np.float32)
    b2 = np.asarray(inputs["b2"], np.float32)
    Wl = np.asarray(inputs["Wl"], np.float32)
    bl = np.asarray(inputs["bl"], np.float32)

    src = np.concatenate([ei[0], np.arange(N, dtype=np.int64)])
    dst = np.concatenate([ei[1], np.arange(N, dtype=np.int64)])
    deg = np.bincount(dst, minlength=N).astype(np.float32)
    dinv = 1.0 / np.sqrt(deg)
    sqdeg = np.sqrt(deg)

    r = _assign(src, dst)            # new id per original node
    inv = np.empty(N, np.int64)
    inv[r] = np.arange(N)            # original node per new id

    # per-(new) node arrays
    dinv_n = dinv[inv]
    sqdeg_n = sqdeg[inv]
    batch_n = batch[inv]
    table0 = (dinv[:, None] * x)[inv].astype(np.float16)  # [N, F] new order

    rs, rd = r[src], r[dst]
    k = rd // C
    shared = {
        "table0": table0,
        "iota128": np.tile(np.arange(128, dtype=np.float16), (128, 1)),
        "giota512": np.tile(np.arange(512, dtype=np.float16), (128, 1)),
        "ones_row": np.ones((1, 128), np.float32),
        "w1": W1.astype(np.float16),
        "w2": W2.astype(np.float16),
        "b1r": b1[None, :].astype(np.float16),
        "b2r": b2[None, :].astype(np.float16),
        "wl": Wl.astype(np.float32),
        "blr": bl[None, :].astype(np.float32),
    }

    per_core = []
    for kk in range(P):
        sel = k == kk
        ms, md = rs[sel], rd[sel] - kk * C   # md in [0, C)
        w = md // 128
        cl = ms % CLS
        li = ms // CLS                        # gather idx within class view
        dl = md - 128 * w                     # dst slot within window

        # slot layout: class stream c, window-major cells of CAP slots
        order = np.lexsort((md, w * CLS + cl))  # group by (w, c); md minor (any)
        w_o, cl_o, li_o, md_o = w[order], cl[order], li[order], md[order]
        cellcnt = np.bincount(w_o * CLS + cl_o, minlength=WPC * CLS).reshape(WPC, CLS)
        if cellcnt.max() > CAP:
            raise RuntimeError("cell overflow after assignment")

        gidx = np.zeros((CLS, CLSIDX), np.int16)            # pad idx 0
        dabs = np.full((CLS, CLSIDX), -1, np.int64)         # absolute dst_local; pad -1
        # place each (w, c) run at offset w*CAP in class stream c
        pos = 0
        for wi in range(WPC):
            for ci in range(CLS):
                n_ = cellcnt[wi, ci]
                seg = slice(pos, pos + n_)
                gidx[ci, wi * CAP : wi * CAP + n_] = li_o[seg].astype(np.int16)
                dabs[ci, wi * CAP : wi * CAP + n_] = md_o[seg]
                pos += n_
        assert pos == ms.shape[0]

        # wrapped gather idx, class blocks concatenated: [128, CLS*CLSC]
        gw = np.concatenate([_wrap_idx(gidx[ci]) for ci in range(CLS)], axis=1)

        # dstloc columns ordered by (w, c, target-tile): [128, NTGT]
        # value = dst_local - 128*w (window-relative); pads and slots of other
        # cells fall outside [0, 128) and never match the iota.
        dcols = np.empty((128, NTGT), np.float16)
        for wi in range(WPC):
            tiles = _tiles_of_window(wi)
            for ci in range(CLS):
                for ti, t in enumerate(tiles):
                    sl = dabs[ci, t * 128 : (t + 1) * 128]
                    rel = np.where(sl < 0, -1, sl - 128 * wi)
                    dcols[:, wi * TPW + ci * TGTW + ti] = rel.astype(np.float16)

        def cols(vals, pad):
            v = np.full(CPAD, pad, vals.dtype)
            v[:C] = vals
            return v.reshape(WPC, 128).T.copy()

        per_core.append(
            {
                "gidx": gw,
                "dstloc": dcols,
                "dinv_c": cols(dinv_n[kk * C : (kk + 1) * C], np.float32(0)),
                "dinv2_c": cols((dinv_n * dinv_n)[kk * C : (kk + 1) * C], np.float32(0)),
                "sqdeg_r": np.concatenate(
                    [sqdeg_n[kk * C : (kk + 1) * C], np.zeros(CPAD - C)]
                ).astype(np.float16)[None, :],
                "bloc_c": cols(
                    batch_n[kk * C : (kk + 1) * C].astype(np.float16),
                    np.float16(-1),
                ),
            }
        )
    return shared, per_core


def _build_program():
    if "nc" in _cache:
        return _cache["nc"]
    import os
    scope = os.environ.get("GCN_SCOPE", "full")  # full | nocoll | l1 | gather
    import concourse.bacc as bacc
    import concourse.mybir as mybir
    import concourse.tile as tile
    from concourse.bass import AP

    f16 = mybir.dt.float16
    f32 = mybir.dt.float32
    i16 = mybir.dt.int16

    nq = int(os.environ.get("GCN_NQ", "4"))
    nc = bacc.Bacc(
        "TRN2", target_bir_lowering=False, debug=False, num_devices=P,
        num_swdge_queues=nq,
    )

    table0_d = nc.dram_tensor("table0", [N, F], f16, kind="ExternalInput")
    gidx_d = nc.dram_tensor("gidx", [128, CLS * CLSC], i16, kind="ExternalInput")
    dstloc_d = nc.dram_tensor("dstloc", [128, NTGT], f16, kind="ExternalInput")
    dinv_d = nc.dram_tensor("dinv_c", [128, WPC], f32, kind="ExternalInput")
    dinv2_d = nc.dram_tensor("dinv2_c", [128, WPC], f32, kind="ExternalInput")
    sqdeg_d = nc.dram_tensor("sqdeg_r", [1, CPAD], f16, kind="ExternalInput")
    bloc_d = nc.dram_tensor("bloc_c", [128, WPC], f16, kind="ExternalInput")
    iota_d = nc.dram_tensor("iota128", [128, 128], f16, kind="ExternalInput")
    giota_d = nc.dram_tensor("giota512", [128, 512], f16, kind="ExternalInput")
    ones_d = nc.dram_tensor("ones_row", [1, 128], f32, kind="ExternalInput")
    w1_d = nc.dram_tensor("w1", [F, F], f16, kind="ExternalInput")
    w2_d = nc.dram_tensor("w2", [F, F], f16, kind="ExternalInput")
    b1_d = nc.dram_tensor("b1r", [1, F], f16, kind="ExternalInput")
    b2_d = nc.dram_tensor("b2r", [1, F], f16, kind="ExternalInput")
    wl_d = nc.dram_tensor("wl", [F, OUT], f32, kind="ExternalInput")
    bl_d = nc.dram_tensor("blr", [1, OUT], f32, kind="ExternalInput")

    t1loc = nc.dram_tensor("t1loc", [C, F], f16)
    # NOTE: t1full must be ordinary DRAM. addr_space="Shared" works for the
    # AllGather but makes the random-access gather reads ~8x slower.
    t1full = nc.dram_tensor("t1full", [N, F], f16)
    poolb = nc.dram_tensor("poolb", [128, G], f32)
    poolr = nc.dram_tensor("poolr", [128, G], f32, addr_space="Shared")
    out_d = nc.dram_tensor("out", [G, OUT], f32, kind="ExternalOutput")

    relu = mybir.ActivationFunctionType.Relu
    iseq = mybir.AluOpType.is_equal
    _deep = int(os.environ.get("GCN_DEEP", "1"))

    with tile.TileContext(nc) as tc:
        with (
            tc.tile_pool(name="const", bufs=1) as cst,
            tc.tile_pool(name="idx", bufs=2 + 2 * _deep) as idxp,
            tc.tile_pool(name="msg", bufs=int(os.environ.get("GCN_MSGBUFS", "3"))) as msgp,
            tc.tile_pool(name="oh", bufs=3 + _deep) as ohp,
            tc.tile_pool(name="small", bufs=3) as smp,
            tc.tile_pool(name="aggps", bufs=2 + _deep, space="PSUM") as aggp,
            tc.tile_pool(name="trps", bufs=2, space="PSUM") as trp,
            tc.tile_pool(name="poolps", bufs=1, space="PSUM") as plp,
        ):
            def load_const(name, dram, shape, dt):
                t = cst.tile(shape, dt, tag=name)
                nc.sync.dma_start(out=t[:], in_=dram[:])
                return t

            iota_t = load_const("iota", iota_d, [128, 128], f16)
            giota_t = load_const("giota", giota_d, [128, 512], f16)
            dstloc_t = load_const("dstloc", dstloc_d, [128, NTGT], f16)
            dinv_t = load_const("dinv", dinv_d, [128, WPC], f32)
            dinv2_t = load_const("dinv2", dinv2_d, [128, WPC], f32)
            sqdeg_t = load_const("sqdeg", sqdeg_d, [1, CPAD], f16)
            bloc_t = load_const("bloc", bloc_d, [128, WPC], f16)
            ones_t = load_const("ones", ones_d, [1, 128], f32)
            w1_t = load_const("w1", w1_d, [F, F], f16)
            w2_t = load_const("w2", w2_d, [F, F], f16)
            b1_t = load_const("b1", b1_d, [1, F], f16)
            b2_t = load_const("b2", b2_d, [1, F], f16)
            wl_t = load_const("wl", wl_d, [F, OUT], f32)
            bl_t = load_const("bl", bl_d, [1, OUT], f32)

            n_repeat = int(os.environ.get("GCN_REPEAT", "1"))
            n_layers = 1 if scope in ("l1", "gather") else 2
            for _rep in range(n_repeat):
              pool_ps = plp.tile([128, G], f32, tag="poolps")
              for L in range(n_layers):
                table = table0_d if L == 0 else t1full
                Wt = w1_t if L == 0 else w2_t
                bt = b1_t if L == 0 else b2_t
                scale_t = dinv2_t if L == 0 else dinv_t

                for s in range(NSEG):
                    nwin = SEGWS[s]
                    wb = SEGOF[s]
                    nidx = nwin * CAP
                    segt = nidx // 128
                    segc = nidx // 16
                    tbase = wb * CAP // 128
                    split = int(os.environ.get("GCN_SPLIT", "2"))
                    if segt % split or (nidx // split) % 128:
                        split = 1
                    msgs = []
                    for ci in range(CLS):
                        it = idxp.tile([128, segc], i16, tag=f"idx{ci}")
                        cb = ci * CLSC + wb * CAP // 16
                        nc.sync.dma_start(
                            out=it[:], in_=gidx_d[:, cb : cb + segc]
                        )
                        mt = msgp.tile([128, segt, F], f16, tag=f"msg{ci}")
                        view = AP(table, ci * F, [[CLS * F, NPC], [1, F]])
                        ht = segt // split
                        hi = nidx // split
                        for h in range(split):
                            nc.gpsimd.dma_gather(
                                mt[:, h * ht : (h + 1) * ht, :], view,
                                it[:, h * hi // 16 : (h + 1) * hi // 16],
                                hi, hi, F,
                                elem_step=CLS * F, single_packet=False,
                                queue_num=(split * ci + h) % nq,
                            )
                        msgs.append(mt)

                    if scope == "gather":
                        sink = smp.tile([128, 128], f16, tag="sink")
                        nc.vector.tensor_copy(out=sink[:], in_=msgs[0][:, 0, :])
                        continue

                    for wl_ in range(nwin):
                        w = wb + wl_
                        oh = ohp.tile([128, TPW, 128], f16, tag="oh")
                        in0 = dstloc_t[:, w * TPW : (w + 1) * TPW].to_broadcast(
                            [128, TPW, 128]
                        )
                        i_ap = iota_t[:]
                        in1 = AP(
                            i_ap.tensor,
                            i_ap.offset,
                            [list(i_ap.ap[0]), [0, TPW], [1, 128]],
                        )
                        nc.vector.tensor_tensor(out=oh[:], in0=in0, in1=in1, op=iseq)

                        agg = aggp.tile([128, 128], f32, tag="agg")
                        wtiles = _tiles_of_window(w)
                        for ci in range(CLS):
                            for ti, t in enumerate(wtiles):
                                nc.tensor.matmul(
                                    out=agg[:],
                                    lhsT=msgs[ci][:, t - tbase, :],
                                    rhs=oh[:, ci * TGTW + ti, :],
                                    start=(ci == 0 and ti == 0),
                                    stop=(ci == CLS - 1 and ti == TGTW - 1),
                                )
                        aggsb = smp.tile([128, 128], f16, tag="aggsb")
                        nc.scalar.copy(out=aggsb[:], in_=agg[:])

                        tp = trp.tile([128, 128], f32, tag="tp")
                        nc.tensor.matmul(
                            out=tp[:], lhsT=aggsb[:], rhs=Wt[:], start=True, stop=False
                        )
                        nc.tensor.matmul(
                            out=tp[:],
                            lhsT=sqdeg_t[0:1, w * 128 : (w + 1) * 128],
                            rhs=bt[:],
                            start=False,
                            stop=True,
                        )
                        tab = smp.tile([128, 128], f16, tag="tab")
                        nc.scalar.activation(
                            out=tab[:], in_=tp[:], func=relu,
                            scale=scale_t[:, w : w + 1],
                        )
                        if L == 0:
                            rows = LASTW if w == WPC - 1 else 128
                            nc.sync.dma_start(
                                out=t1loc[w * 128 : w * 128 + rows, :],
                                in_=tab[0:rows, :],
                            )
                        else:
                            ohb = ohp.tile([128, G], f16, tag="ohb")
                            nc.vector.tensor_tensor(
                                out=ohb[:],
                                in0=bloc_t[:, w : w + 1].to_broadcast([128, G]),
                                in1=giota_t[:],
                                op=iseq,
                            )
                            nc.tensor.matmul(
                                out=pool_ps[:],
                                lhsT=tab[:],
                                rhs=ohb[:],
                                start=(w == 0),
                                stop=(w == WPC - 1),
                            )
                if L == 0 and n_layers == 2:
                    if scope == "full":
                        nc.gpsimd.collective_compute(
                            "AllGather",
                            mybir.AluOpType.bypass,
                            replica_groups=[list(range(P))],
                            ins=[t1loc[:]],
                            outs=[t1full[:]],
                        )
                    else:
                        nc.gpsimd.dma_start(out=t1full[0:C, :], in_=t1loc[:])

            if scope in ("l1", "gather"):
                zt = smp.tile([128, OUT], f32, tag="zt")
                nc.vector.memset(zt[:], 0.0)
                for gs in range(G // 128):
                    nc.sync.dma_start(
                        out=out_d[gs * 128 : (gs + 1) * 128, :], in_=zt[:]
                    )
            else:
                poolsb = smp.tile([128, G], f32, tag="poolsb")
                nc.scalar.copy(out=poolsb[:], in_=pool_ps[:])
                nc.gpsimd.dma_start(out=poolb[:], in_=poolsb[:])
                if scope == "full":
                    nc.gpsimd.collective_compute(
                        "AllReduce",
                        mybir.AluOpType.add,
                        replica_groups=[list(range(P))],
                        ins=[poolb[:]],
                        outs=[poolr[:]],
                    )
                else:
                    nc.gpsimd.dma_start(out=poolr[:], in_=poolb[:])
                prsb = smp.tile([128, G], f32, tag="prsb")
                nc.sync.dma_start(out=prsb[:], in_=poolr[:])
                for gs in range(G // 128):
                    fps = trp.tile([128, OUT], f32, tag="fps")
                    nc.tensor.matmul(
                        out=fps[:],
                        lhsT=prsb[:, gs * 128 : (gs + 1) * 128],
                        rhs=wl_t[:],
                        start=True,
                        stop=False,
                    )
                    nc.tensor.matmul(
                        out=fps[:], lhsT=ones_t[0:1, :], rhs=bl_t[:],
                        start=False, stop=True,
                    )
                    osb = smp.tile([128, OUT], f32, tag="osb")
                    nc.scalar.copy(out=osb[:], in_=fps[:])
                    nc.sync.dma_start(
                        out=out_d[gs * 128 : (gs + 1) * 128, :], in_=osb[:]
                    )

    nc.compile()
    _cache["nc"] = nc
    return nc


def kernel(**inputs):
    from concourse.bass_utils import run_bass_kernel_spmd

    shared, per_core = _preprocess(inputs)
    nc = _build_program()
    in_maps = [{**shared, **pc} for pc in per_core]
    res = run_bass_kernel_spmd(nc, in_maps, list(range(P))).results
    return res[0]["out"].astype(np.float32)
